# revision 20
# baseline (speedup 1.0000x reference)
"""Multi-head attention (B=4, N=2048, C=768, H=12) on 8 TRN2 NeuronCores.

Sharding: 4 batches x 2 head-groups (6 heads each); core = 2*b + g.

Structure (v2): the kernel is ACT(exp)-bound -- 192 softmax tiles of
[128,1024] at ~1.11us each.  Everything else (qkv GEMMs, V, output
projection, collectives) is emitted as rate-limited "filler" chains pumped
into the attention slot loop so the PE's idle time absorbs them and the
ACT never starves.  Chains are demand-flushed (need()) before any consumer
instruction, so emission (program) order always respects data flow; the
pump is pacing only.  A fraction of the exp tiles can be offloaded to the
DVE via a Schraudolph int16 bit-trick (exp(x) ~= bitcast_bf16(int16(A*x+B)))
which runs concurrently with the ACT.

Per core:
  - qT/kT [64,2048] per head and v [2048,64] per head from host-pre-transposed xT
  - flash-style attention on transposed-S tiles:
      S^T(m,n) = kT.T @ qT   (PE, bf16, two heads in disjoint row groups)
      P^T = exp(S^T/8)       (ACT exp or DVE int16-Schraudolph, -> bf16)
      o^T = [v|1].T @ P^T    (PE; ones column accumulates softmax row-sums)
  - normalize columns of o^T via reciprocal + K=1 broadcast matmul
  - AllGather of normalized aoT (bf16) between pair cores in three column
    groups (1024/512/512) so the collectives overlap remaining attention
  - each core projects the gathered aoT onto its w_out column slice
Host only concatenates the 8 column-slices (no host math).
"""

import math
import sys
from collections import deque

sys.path.insert(0, "/opt/trn_rl_repo")

import ml_dtypes
import numpy as np

import concourse.bass as bass
import concourse.mybir as mybir
from concourse import bacc, tile
from concourse.bass_utils import run_bass_kernel_spmd

F32 = mybir.dt.float32
BF16 = mybir.dt.bfloat16
I16 = mybir.dt.int16

B, N, C, H, D = 4, 2048, 768, 12, 64
G = 2               # head groups (tensor-parallel dim)
HPC = H // G        # heads per core = 6
KC = HPC * D        # per-core head width = 384
CT = C // 128       # contraction tiles over C = 6
NT = N // 128       # 128-row seq tiles = 16
SCALE = D ** -0.5

# Schraudolph exp-in-bf16-bits: exp(s*SCALE) ~= bitcast_bf16(int16(A*s + B)).
# C16 tuned offline (numpy sim: rel err 1.2e-2 at 25% offload vs 5.3e-3 pure;
# gate is 2e-2).
_C16 = 5.5
EXP_A = (128.0 / math.log(2.0)) * SCALE
EXP_B = 127.0 * 128.0 - _C16
# mj slots (0..15 within each chunk) evacuated on DVE instead of ACT
DVE_SLOTS = frozenset()  # phase 1 validation: ACT-only
DVE_FROM_CHUNK = 1  # skip chunk 0 (DVE busy with v casts / qk fillers)

# chunk list: (hp, col, ag_idx); col = 512-wide q-column base
CHUNKS = [
    (0, 0, 0), (0, 512, 0), (1, 0, 0), (1, 512, 0), (2, 0, 0), (2, 512, 0),
    (0, 1024, 1), (1, 1024, 1), (2, 1024, 1),
    (0, 1536, 2), (1, 1536, 2), (2, 1536, 2),
]
AG_AFTER = {5: 0, 8: 1, 11: 2}          # chunk idx -> ag group emitted after
AG_BASE = {0: 0, 1: 1024, 2: 1536}      # ag group -> q-column base
AG_W = {0: 1024, 1: 512, 2: 512}        # ag group -> width
AG_NJL = {0: range(0, 8), 1: range(8, 12), 2: range(12, 16)}  # y 128-row blocks


class Chain:
    """A short sequence of emission steps with demand-flush semantics."""

    __slots__ = ("steps", "idx")

    def __init__(self, steps):
        self.steps = steps
        self.idx = 0

    def emit_next(self):
        if self.idx >= len(self.steps):
            return None
        cost, fn = self.steps[self.idx]
        self.idx += 1
        fn()
        return cost

    def flush(self):
        while self.emit_next() is not None:
            pass


def _build():
    nc = bacc.Bacc(None, num_devices=8)

    xT_d = nc.declare_dram_parameter("xT", [C, N], BF16, isOutput=False)
    wq_d = nc.declare_dram_parameter("wq", [C, KC], BF16, isOutput=False)
    wk_d = nc.declare_dram_parameter("wk", [C, KC], BF16, isOutput=False)
    wv_d = nc.declare_dram_parameter("wv", [C, KC], BF16, isOutput=False)
    wo_d = nc.declare_dram_parameter("wo", [C, KC], BF16, isOutput=False)
    bb_d = nc.declare_dram_parameter("bb", [128, KC], F32, isOutput=False)
    y_d = nc.declare_dram_parameter("y", [N, KC], BF16, isOutput=True)

    with tile.TileContext(nc) as tc:
        with (
            tc.tile_pool(name="wpool", bufs=1) as wpool,
            tc.tile_pool(name="work", bufs=3) as work,
            tc.tile_pool(name="psum", bufs=2, space="PSUM") as psum,
            tc.tile_pool(name="dram", bufs=1, space="DRAM") as dram,
        ):
            xpool = seq = wpool
            small = work
            # ---- SBUF residents ----
            wq_sb = wpool.tile([128, CT, KC], BF16)
            wk_sb = wpool.tile([128, CT, KC], BF16)
            wv_sb = wpool.tile([128, CT, KC], BF16)
            wo_sb = wpool.tile([128, CT, KC], BF16)
            bb_sb = wpool.tile([128, KC], F32)
            xT_sb = xpool.tile([128, CT, N], BF16)
            qT_sb = [seq.tile([128, N], BF16, name=f"qT{t}", tag=f"qT{t}") for t in range(3)]
            kT_sb = [seq.tile([128, N], BF16, name=f"kT{t}", tag=f"kT{t}") for t in range(3)]
            v_sb = seq.tile([128, NT * HPC * 65], BF16, tag="v")
            ao_sb = [seq.tile([128, N], BF16, name=f"ao{t}", tag=f"ao{t}") for t in range(3)]
            ones_sb = small.tile([1, 64], BF16, bufs=1)

            with nc.named_scope("load"):
                # Single 3D DMAs per tensor on two parallel rings: the
                # prologue's k/q/v ni=0 GEMMs gate the first attention slot.
                for ct in range(CT):
                    nc.sync.dma_start(wk_sb[:, ct, :], wk_d[ct * 128:(ct + 1) * 128, :])
                    nc.sync.dma_start(wq_sb[:, ct, :], wq_d[ct * 128:(ct + 1) * 128, :])
                    nc.gpsimd.dma_start(xT_sb[:, ct, 0:512], xT_d[ct * 128:(ct + 1) * 128, 0:512])
                for ct in range(CT):
                    nc.gpsimd.dma_start(wv_sb[:, ct, :], wv_d[ct * 128:(ct + 1) * 128, :])
                for ct in range(CT):
                    nc.gpsimd.dma_start(
                        xT_sb[:, ct, 512:1024], xT_d[ct * 128:(ct + 1) * 128, 512:1024])
                for ni in range(2, 4):
                    for ct in range(CT):
                        nc.sync.dma_start(
                            xT_sb[:, ct, ni * 512:(ni + 1) * 512],
                            xT_d[ct * 128:(ct + 1) * 128, ni * 512:(ni + 1) * 512],
                        )
                for ct in range(CT):
                    nc.sync.dma_start(wo_sb[:, ct, :], wo_d[ct * 128:(ct + 1) * 128, :])
                nc.sync.dma_start(bb_sb[:], bb_d[:])

            # ones column at offset 64 of every 65-wide v block (row-sum trick)
            nc.vector.memset(v_sb.rearrange("p (b s) -> p b s", s=65)[:, :, 64], 1.0)
            nc.vector.memset(ones_sb[:], 1.0)

            ag_in = [dram.tile([KC, AG_W[g]], BF16, name=f"ag_in{g}") for g in range(3)]
            ag_out = [dram.tile([C, AG_W[g]], BF16, name=f"ag_out{g}") for g in range(3)]

            # ---------------- filler chains ----------------
            # The deque paces opportunistic emission into PE slack; need()
            # force-flushes a producer chain before its consumer is emitted.
            fillers = deque()
            pump_state = {'credit': 0}

            def pump(budget=400):
                credit = min(pump_state['credit'] + budget, 1000)
                while fillers and credit > 0:
                    cost = fillers[0].emit_next()
                    if cost is None:
                        fillers.popleft()
                        continue
                    credit -= cost
                pump_state['credit'] = credit if fillers else 0

            def flush_fillers():
                while fillers:
                    fillers.popleft().flush()

            def qk_chain(wsb, dst, hp, ni):
                """3 steps; projects 512 seq-cols of q or k for head-pair hp."""
                st = {}
                def mk(cts, first, last):
                    def s():
                        if first:
                            st['ps'] = psum.tile([128, 512], F32, name="qk_ps", tag="mm")
                        for ct in cts:
                            nc.tensor.matmul(
                                st['ps'][:], wsb[:, ct, hp * 128:(hp + 1) * 128],
                                xT_sb[:, ct, ni * 512:(ni + 1) * 512],
                                start=(first and ct == cts[0]), stop=(last and ct == cts[-1]))
                        if last:
                            nc.vector.tensor_copy(dst[:, ni * 512:(ni + 1) * 512], st['ps'][:])
                    return s
                return Chain([(440, mk((0, 1), True, False)),
                              (440, mk((2, 3), False, False)),
                              (470, mk((4, 5), False, True))])

            def v_chain(mj):
                """3 steps; v rows for seq-tile mj, all 6 heads (65-strided)."""
                st = {}
                def mk(cts, first, last):
                    def s():
                        if first:
                            st['ps'] = psum.tile([128, KC], F32, name="v_ps", tag="mm")
                        for ct in cts:
                            nc.tensor.matmul(
                                st['ps'][:], xT_sb[:, ct, mj * 128:(mj + 1) * 128],
                                wv_sb[:, ct, :],
                                start=(first and ct == cts[0]), stop=(last and ct == cts[-1]))
                        if last:
                            dst = v_sb.rearrange("p (b s) -> p b s", s=65)[
                                :, mj * HPC:(mj + 1) * HPC, 0:64]
                            src = st['ps'][:].rearrange("p (h d) -> p h d", d=64)
                            nc.vector.tensor_copy(dst, src)
                    return s
                return Chain([(340, mk((0, 1), True, False)),
                              (340, mk((2, 3), False, False)),
                              (370, mk((4, 5), False, True))])

            aoF = [work.tile([128, 1024], BF16, name=f"aoF{kt}", tag=f"aoF{kt}", bufs=1)
                   for kt in range(CT)]

            def proj_chain(g, njls=None, load=True, fast_load=False):
                """aoF loads + y projection for ag group g (after its AllGather)."""
                w = AG_W[g]
                steps = []
                if load:
                    for kt in range(CT):
                        def sdma(kt=kt):
                            # fast_load: split across two idle queues (epilogue)
                            eng = nc.gpsimd if (fast_load and kt % 2) else nc.sync
                            eng.dma_start(aoF[kt][:, 0:w], ag_out[g][kt * 128:(kt + 1) * 128, :])
                        steps.append((30, sdma))
                for nj in (AG_NJL[g] if njls is None else njls):
                    cbase = nj * 128 - AG_BASE[g]
                    st = {}
                    def mk(kts, first, last, nj=nj, cbase=cbase, st=st):
                        def s():
                            if first:
                                st['ps'] = psum.tile([128, KC], F32, name="y_ps", tag="mm")
                            for kt in kts:
                                nc.tensor.matmul(
                                    st['ps'][:], aoF[kt][:, cbase:cbase + 128],
                                    wo_sb[:, kt, :],
                                    start=(first and kt == kts[0]),
                                    stop=(last and kt == kts[-1]))
                            if last:
                                y_sb = work.tile([128, KC], BF16, name="y_sb", tag="y")
                                nc.vector.tensor_add(y_sb[:], st['ps'][:], bb_sb[:])
                                nc.sync.dma_start(y_d[nj * 128:(nj + 1) * 128, :], y_sb[:])
                        return s
                    steps += [(510, mk((0, 1, 2), True, False)),
                              (540, mk((3, 4, 5), False, True))]
                return Chain(steps)

            # chain registries for demand flushing
            k_ch = {}
            q_ch = {}
            v_ch = {}
            for hp in range(3):
                for ni in range(4):
                    k_ch[(hp, ni)] = qk_chain(wk_sb, kT_sb[hp], hp, ni)
                    q_ch[(hp, ni)] = qk_chain(wq_sb, qT_sb[hp], hp, ni)
            for mj in range(NT):
                v_ch[mj] = v_chain(mj)

            # ---------------- attention emitters ----------------
            sT_live = {}
            pT_live = {}

            def emit_S(hp, col, mj):
                # producers first (program order == semantics)
                k_ch[(hp, mj // 4)].flush()
                q_ch[(hp, col // 512)].flush()
                sT = psum.tile([128, 1024], F32, name="sT", tag="sT", bufs=2)
                sT_live[(hp, col, mj)] = sT
                for i in range(2):
                    po = i * 64
                    nc.tensor.matmul(
                        sT[:, i * 512:(i + 1) * 512],
                        kT_sb[hp][po:po + 64, mj * 128:(mj + 1) * 128],
                        qT_sb[hp][po:po + 64, col:col + 512],
                        start=True, stop=True)

            def emit_evac(ci, hp, col, mj):
                sT = sT_live.pop((hp, col, mj))
                pT = work.tile([128, 1024], BF16, name="pT", tag="pT", bufs=16)
                pT_live[(hp, col, mj)] = pT
                if ci >= DVE_FROM_CHUNK and mj in DVE_SLOTS:
                    nc.vector.tensor_scalar(
                        pT[:].bitcast(I16), sT[:], float(EXP_A), float(EXP_B),
                        mybir.AluOpType.mult, mybir.AluOpType.add)
                else:
                    nc.scalar.activation(
                        pT[:], sT[:], mybir.ActivationFunctionType.Exp, scale=SCALE)

            def emit_O(hp, col, mj, oT):
                v_ch[mj].flush()
                pT = pT_live.pop((hp, col, mj))
                for i in range(2):
                    h = hp * 2 + i
                    vblk = v_sb[:, (mj * HPC + h) * 65:(mj * HPC + h) * 65 + 65]
                    nc.tensor.matmul(
                        oT[i][:], vblk, pT[:, i * 512:(i + 1) * 512],
                        start=(mj == 0), stop=(mj == NT - 1))

            def norm_head(hp, col, oT, rinvs):
                # free the oT psum banks fast: copy out + row-sum reciprocal
                for i in range(2):
                    po = i * 64
                    ao_slice = ao_sb[hp][po:po + 64, col:col + 512]
                    nc.vector.tensor_copy(ao_slice, oT[i][0:64, :])
                    r_row = small.tile([1, 512], F32, name="r_row", tag="r_row", bufs=4)
                    nc.vector.tensor_copy(r_row[:], oT[i][64:65, :])
                    rinv = small.tile([1, 512], F32, name="rinv", tag="rinv", bufs=4)
                    nc.vector.reciprocal_approx_fast(rinv[:], r_row[:])
                    rinvs.append(rinv)

            def norm_tail_chain(hp, col, g, rinvs):
                steps = []
                for i in range(2):
                    po = i * 64
                    rinv = rinvs[i]
                    st = {}
                    def s1(rinv=rinv, st=st):
                        rb_row = small.tile([1, 512], BF16, name="rb_row", tag="rb_row", bufs=4)
                        nc.vector.tensor_copy(rb_row[:], rinv[:])
                        rb_ps = psum.tile([64, 512], F32, name="rb_ps", tag="mm")
                        st['rb_ps'] = rb_ps
                        nc.tensor.matmul(rb_ps[:], ones_sb[:], rb_row[:], start=True, stop=True)
                    def s2(po=po, st=st):
                        ao_slice = ao_sb[hp][po:po + 64, col:col + 512]
                        nc.vector.tensor_mul(ao_slice, ao_slice, st['rb_ps'][:])
                        nc.gpsimd.dma_start(
                            ag_in[g][hp * 128 + po:hp * 128 + po + 64,
                                     col - AG_BASE[g]:col - AG_BASE[g] + 512],
                            ao_slice)
                    steps += [(240, s1), (30, s2)]
                return Chain(steps)

            # ---------------- prologue ----------------
            with nc.named_scope("qkv"):
                ka, qa = k_ch[(0, 0)], q_ch[(0, 0)]
                while ka.emit_next() is not None and qa.emit_next() is not None:
                    pass
                ka.flush()
                qa.flush()
                for mj in range(3):
                    v_ch[mj].flush()

            # pacing order for the rest (need() guarantees correctness)
            fillers.extend([k_ch[(0, 1)], v_ch[3], v_ch[4],
                            k_ch[(0, 2)], v_ch[5], v_ch[6],
                            k_ch[(0, 3)], v_ch[7], v_ch[8], v_ch[9],
                            q_ch[(0, 1)],
                            v_ch[10], v_ch[11], v_ch[12], v_ch[13], v_ch[14], v_ch[15]])
            for hp in (1, 2):
                for ni in range(4):
                    fillers.append(k_ch[(hp, ni)])
                for ni in range(4):
                    fillers.append(q_ch[(hp, ni)])
            fillers.append(q_ch[(0, 2)])
            fillers.append(q_ch[(0, 3)])

            # ---------------- main chunk loop ----------------
            group_tails = {0: [], 1: [], 2: []}
            pending_tail = None
            delayed = {}
            for ci, (hp, col, g) in enumerate(CHUNKS):
                with nc.named_scope(f"attn{ci}"):
                    if ci == 0:
                        emit_S(hp, col, 0)
                        emit_S(hp, col, 1)
                    if pending_tail is not None:
                        fillers.appendleft(pending_tail)
                        pending_tail = None
                    for ch in delayed.pop(ci, ()):
                        fillers.append(ch)
                    oT = [psum.tile([65, 512], F32, name=f"oT{i}", tag="oT")
                          for i in range(2)]
                    for mj in range(NT):
                        emit_evac(ci, hp, col, mj)
                        if mj + 2 < NT:
                            emit_S(hp, col, mj + 2)
                        elif ci + 1 < len(CHUNKS):
                            nhp, ncol, _ = CHUNKS[ci + 1]
                            emit_S(nhp, ncol, mj + 2 - NT)
                        emit_O(hp, col, mj, oT)
                        pump()
                    rinvs = []
                    norm_head(hp, col, oT, rinvs)
                    tail = norm_tail_chain(hp, col, g, rinvs)
                    group_tails[g].append(tail)
                    if ci in AG_AFTER:
                        gi = AG_AFTER[ci]
                        for ch in group_tails[gi]:
                            ch.flush()   # all ships of this group before the AG
                        with nc.named_scope(f"ag{gi}"):
                            nc.gpsimd.collective_compute(
                                "AllGather",
                                mybir.AluOpType.bypass,
                                replica_groups=[[0, 1], [2, 3], [4, 5], [6, 7]],
                                ins=[ag_in[gi].opt()],
                                outs=[ag_out[gi].opt()],
                            )
                        if gi == 0:
                            # pump during chunks ci+2.. (AG latency ~9us must
                            # pass before the aoF loads hit the sync queue)
                            delayed.setdefault(ci + 2, []).append(proj_chain(0))
                        elif gi == 1:
                            # njl 10,11 held back: their MMs keep the PE warm
                            # during the final AllGather's latency window
                            delayed.setdefault(ci + 2, []).append(
                                proj_chain(1, njls=range(8, 9)))
                    else:
                        pending_tail = tail

            # ---------------- epilogue ----------------
            with nc.named_scope("proj2"):
                flush_fillers()
                # group-1 leftovers execute while AG2 is in flight (PE warm)
                proj_chain(1, njls=range(9, 12), load=False).flush()
                proj_chain(2, fast_load=True).flush()

    nc.finalize()
    return nc


_NC = None
LAST_RESULTS = None


def _get_nc():
    global _NC
    if _NC is None:
        _NC = _build()
    return _NC


def kernel(x, w_qkv, w_out, b_out, _trace=False):
    global LAST_RESULTS
    nc = _get_nc()

    x = np.asarray(x, dtype=np.float32)
    w_qkv = np.asarray(w_qkv, dtype=np.float32)
    w_out = np.asarray(w_out, dtype=np.float32)
    b_out = np.asarray(b_out, dtype=np.float32)

    bf16 = ml_dtypes.bfloat16
    in_maps = []
    for c in range(8):
        b, g = c // 2, c % 2
        s = g * KC
        in_maps.append({
            "xT": np.ascontiguousarray(x[b].T).astype(bf16),
            "wq": np.ascontiguousarray(w_qkv[:, s:s + KC]).astype(bf16),
            "wk": np.ascontiguousarray(w_qkv[:, C + s:C + s + KC]).astype(bf16),
            "wv": np.ascontiguousarray(w_qkv[:, 2 * C + s:2 * C + s + KC]).astype(bf16),
            "wo": np.ascontiguousarray(w_out[:, s:s + KC]).astype(bf16),
            "bb": np.tile(b_out[s:s + KC], (128, 1)),
        })

    res = run_bass_kernel_spmd(nc, in_maps, core_ids=list(range(8)), trace=_trace)
    LAST_RESULTS = res

    out = np.empty((B, N, C), dtype=np.float32)
    for c in range(8):
        b, g = c // 2, c % 2
        out[b, :, g * KC:(g + 1) * KC] = res.results[c]["y"].astype(np.float32)
    return out


# revision 21
# speedup vs baseline: 1.0129x; 1.0129x over previous
"""Multi-head attention (B=4, N=2048, C=768, H=12) on 8 TRN2 NeuronCores.

Sharding: 4 batches x 2 head-groups (6 heads each); core = 2*b + g.

Structure (v2): the kernel is ACT(exp)-bound -- 192 softmax tiles of
[128,1024] at ~1.11us each.  Everything else (qkv GEMMs, V, output
projection, collectives) is emitted as rate-limited "filler" chains pumped
into the attention slot loop so the PE's idle time absorbs them and the
ACT never starves.  Chains are demand-flushed (need()) before any consumer
instruction, so emission (program) order always respects data flow; the
pump is pacing only.  A fraction of the exp tiles can be offloaded to the
DVE via a Schraudolph int16 bit-trick (exp(x) ~= bitcast_bf16(int16(A*x+B)))
which runs concurrently with the ACT.

Per core:
  - qT/kT [64,2048] per head and v [2048,64] per head from host-pre-transposed xT
  - flash-style attention on transposed-S tiles:
      S^T(m,n) = kT.T @ qT   (PE, bf16, two heads in disjoint row groups)
      P^T = exp(S^T/8)       (ACT exp or DVE int16-Schraudolph, -> bf16)
      o^T = [v|1].T @ P^T    (PE; ones column accumulates softmax row-sums)
  - normalize columns of o^T via reciprocal + K=1 broadcast matmul
  - AllGather of normalized aoT (bf16) between pair cores in three column
    groups (1024/512/512) so the collectives overlap remaining attention
  - each core projects the gathered aoT onto its w_out column slice
Host only concatenates the 8 column-slices (no host math).
"""

import math
import sys
from collections import deque

sys.path.insert(0, "/opt/trn_rl_repo")

import ml_dtypes
import numpy as np

import concourse.bass as bass
import concourse.mybir as mybir
from concourse import bacc, tile
from concourse.bass_utils import run_bass_kernel_spmd

F32 = mybir.dt.float32
BF16 = mybir.dt.bfloat16
I16 = mybir.dt.int16

B, N, C, H, D = 4, 2048, 768, 12, 64
G = 2               # head groups (tensor-parallel dim)
HPC = H // G        # heads per core = 6
KC = HPC * D        # per-core head width = 384
CT = C // 128       # contraction tiles over C = 6
NT = N // 128       # 128-row seq tiles = 16
SCALE = D ** -0.5

# Schraudolph exp-in-bf16-bits: exp(s*SCALE) ~= bitcast_bf16(int16(A*s + B)).
# C16 tuned offline (numpy sim: rel err 1.2e-2 at 25% offload vs 5.3e-3 pure;
# gate is 2e-2).
_C16 = 5.5
EXP_A = (128.0 / math.log(2.0)) * SCALE
EXP_B = 127.0 * 128.0 - _C16
# mj slots (0..15 within each chunk) evacuated on DVE instead of ACT
DVE_SLOTS = frozenset()  # phase 1 validation: ACT-only
DVE_FROM_CHUNK = 1  # skip chunk 0 (DVE busy with v casts / qk fillers)

# chunk list: (hp, col, ag_idx); col = 512-wide q-column base
CHUNKS = [
    (0, 0, 0), (0, 512, 0), (1, 0, 0), (1, 512, 0), (2, 0, 0), (2, 512, 0),
    (0, 1024, 1), (1, 1024, 1), (2, 1024, 1),
    (0, 1536, 2), (1, 1536, 2), (2, 1536, 2),
]
AG_AFTER = {5: 0, 8: 1, 11: 2}          # chunk idx -> ag group emitted after
AG_BASE = {0: 0, 1: 1024, 2: 1536}      # ag group -> q-column base
AG_W = {0: 1024, 1: 512, 2: 512}        # ag group -> width
AG_NJL = {0: range(0, 8), 1: range(8, 12), 2: range(12, 16)}  # y 128-row blocks


class Chain:
    """A short sequence of emission steps with demand-flush semantics."""

    __slots__ = ("steps", "idx")

    def __init__(self, steps):
        self.steps = steps
        self.idx = 0

    def emit_next(self):
        if self.idx >= len(self.steps):
            return None
        cost, fn = self.steps[self.idx]
        self.idx += 1
        fn()
        return cost

    def flush(self):
        while self.emit_next() is not None:
            pass


def _build():
    nc = bacc.Bacc(None, num_devices=8)

    xT_d = nc.declare_dram_parameter("xT", [C, N], BF16, isOutput=False)
    wq_d = nc.declare_dram_parameter("wq", [C, KC], BF16, isOutput=False)
    wk_d = nc.declare_dram_parameter("wk", [C, KC], BF16, isOutput=False)
    wv_d = nc.declare_dram_parameter("wv", [C, KC], BF16, isOutput=False)
    wo_d = nc.declare_dram_parameter("wo", [C, KC], BF16, isOutput=False)
    bb_d = nc.declare_dram_parameter("bb", [128, KC], F32, isOutput=False)
    y_d = nc.declare_dram_parameter("y", [N, KC], BF16, isOutput=True)

    with tile.TileContext(nc) as tc:
        with (
            tc.tile_pool(name="wpool", bufs=1) as wpool,
            tc.tile_pool(name="work", bufs=3) as work,
            tc.tile_pool(name="psum", bufs=2, space="PSUM") as psum,
            tc.tile_pool(name="dram", bufs=1, space="DRAM") as dram,
        ):
            xpool = seq = wpool
            small = work
            # ---- SBUF residents ----
            wq_sb = wpool.tile([128, CT, KC], BF16)
            wk_sb = wpool.tile([128, CT, KC], BF16)
            wv_sb = wpool.tile([128, CT, KC], BF16)
            wo_sb = wpool.tile([128, CT, KC], BF16)
            bb_sb = wpool.tile([128, KC], F32)
            xT_sb = xpool.tile([128, CT, N], BF16)
            qT_sb = [seq.tile([128, N], BF16, name=f"qT{t}", tag=f"qT{t}") for t in range(3)]
            kT_sb = [seq.tile([128, N], BF16, name=f"kT{t}", tag=f"kT{t}") for t in range(3)]
            v_sb = seq.tile([128, NT * HPC * 65], BF16, tag="v")
            ao_sb = [seq.tile([128, N], BF16, name=f"ao{t}", tag=f"ao{t}") for t in range(3)]
            ones_sb = small.tile([1, 64], BF16, bufs=1)

            with nc.named_scope("load"):
                # Single 3D DMAs per tensor on two parallel rings: the
                # prologue's k/q/v ni=0 GEMMs gate the first attention slot.
                for ct in range(CT):
                    nc.sync.dma_start(wk_sb[:, ct, :], wk_d[ct * 128:(ct + 1) * 128, :])
                    nc.sync.dma_start(wq_sb[:, ct, :], wq_d[ct * 128:(ct + 1) * 128, :])
                    nc.gpsimd.dma_start(xT_sb[:, ct, 0:512], xT_d[ct * 128:(ct + 1) * 128, 0:512])
                for ct in range(CT):
                    nc.gpsimd.dma_start(wv_sb[:, ct, :], wv_d[ct * 128:(ct + 1) * 128, :])
                for ct in range(CT):
                    nc.gpsimd.dma_start(
                        xT_sb[:, ct, 512:1024], xT_d[ct * 128:(ct + 1) * 128, 512:1024])
                for ni in range(2, 4):
                    for ct in range(CT):
                        nc.sync.dma_start(
                            xT_sb[:, ct, ni * 512:(ni + 1) * 512],
                            xT_d[ct * 128:(ct + 1) * 128, ni * 512:(ni + 1) * 512],
                        )
                for ct in range(CT):
                    nc.sync.dma_start(wo_sb[:, ct, :], wo_d[ct * 128:(ct + 1) * 128, :])
                nc.sync.dma_start(bb_sb[:], bb_d[:])

            # ones column at offset 64 of every 65-wide v block (row-sum trick)
            nc.vector.memset(v_sb.rearrange("p (b s) -> p b s", s=65)[:, :, 64], 1.0)
            nc.vector.memset(ones_sb[:], 1.0)

            ag_in = [dram.tile([KC, AG_W[g]], BF16, name=f"ag_in{g}") for g in range(3)]
            ag_out = [dram.tile([C, AG_W[g]], BF16, name=f"ag_out{g}") for g in range(3)]

            # ---------------- filler chains ----------------
            # The deque paces opportunistic emission into PE slack; need()
            # force-flushes a producer chain before its consumer is emitted.
            fillers = deque()
            pump_state = {'credit': 0}

            def pump(budget=400):
                credit = min(pump_state['credit'] + budget, 1000)
                while fillers and credit > 0:
                    cost = fillers[0].emit_next()
                    if cost is None:
                        fillers.popleft()
                        continue
                    credit -= cost
                pump_state['credit'] = credit if fillers else 0

            def flush_fillers():
                while fillers:
                    fillers.popleft().flush()

            def qk_chain(wsb, dst, hp, ni):
                """3 steps; projects 512 seq-cols of q or k for head-pair hp."""
                st = {}
                def mk(cts, first, last):
                    def s():
                        if first:
                            st['ps'] = psum.tile([128, 512], F32, name="qk_ps", tag="mm")
                        for ct in cts:
                            nc.tensor.matmul(
                                st['ps'][:], wsb[:, ct, hp * 128:(hp + 1) * 128],
                                xT_sb[:, ct, ni * 512:(ni + 1) * 512],
                                start=(first and ct == cts[0]), stop=(last and ct == cts[-1]))
                        if last:
                            nc.vector.tensor_copy(dst[:, ni * 512:(ni + 1) * 512], st['ps'][:])
                    return s
                return Chain([(440, mk((0, 1), True, False)),
                              (440, mk((2, 3), False, False)),
                              (470, mk((4, 5), False, True))])

            def v_chain(mj):
                """3 steps; v rows for seq-tile mj, all 6 heads (65-strided)."""
                st = {}
                def mk(cts, first, last):
                    def s():
                        if first:
                            st['ps'] = psum.tile([128, KC], F32, name="v_ps", tag="mm")
                        for ct in cts:
                            nc.tensor.matmul(
                                st['ps'][:], xT_sb[:, ct, mj * 128:(mj + 1) * 128],
                                wv_sb[:, ct, :],
                                start=(first and ct == cts[0]), stop=(last and ct == cts[-1]))
                        if last:
                            dst = v_sb.rearrange("p (b s) -> p b s", s=65)[
                                :, mj * HPC:(mj + 1) * HPC, 0:64]
                            src = st['ps'][:].rearrange("p (h d) -> p h d", d=64)
                            nc.vector.tensor_copy(dst, src)
                    return s
                return Chain([(340, mk((0, 1), True, False)),
                              (340, mk((2, 3), False, False)),
                              (370, mk((4, 5), False, True))])

            aoF = [work.tile([128, 1024], BF16, name=f"aoF{kt}", tag=f"aoF{kt}", bufs=1)
                   for kt in range(CT)]

            def proj_chain(g, njls=None, load=True, fast_load=False):
                """aoF loads + y projection for ag group g (after its AllGather)."""
                w = AG_W[g]
                steps = []
                if load:
                    for kt in range(CT):
                        def sdma(kt=kt):
                            # fast_load: split across two idle queues (epilogue)
                            eng = nc.gpsimd if (fast_load and kt % 2) else nc.sync
                            eng.dma_start(aoF[kt][:, 0:w], ag_out[g][kt * 128:(kt + 1) * 128, :])
                        steps.append((30, sdma))
                for nj in (AG_NJL[g] if njls is None else njls):
                    cbase = nj * 128 - AG_BASE[g]
                    st = {}
                    def mk(kts, first, last, nj=nj, cbase=cbase, st=st):
                        def s():
                            if first:
                                st['ps'] = psum.tile([128, KC], F32, name="y_ps", tag="mm")
                            for kt in kts:
                                nc.tensor.matmul(
                                    st['ps'][:], aoF[kt][:, cbase:cbase + 128],
                                    wo_sb[:, kt, :],
                                    start=(first and kt == kts[0]),
                                    stop=(last and kt == kts[-1]))
                            if last:
                                y_sb = work.tile([128, KC], BF16, name="y_sb", tag="y")
                                nc.vector.tensor_add(y_sb[:], st['ps'][:], bb_sb[:])
                                nc.sync.dma_start(y_d[nj * 128:(nj + 1) * 128, :], y_sb[:])
                        return s
                    steps += [(510, mk((0, 1, 2), True, False)),
                              (540, mk((3, 4, 5), False, True))]
                return Chain(steps)

            # chain registries for demand flushing
            k_ch = {}
            q_ch = {}
            v_ch = {}
            for hp in range(3):
                for ni in range(4):
                    k_ch[(hp, ni)] = qk_chain(wk_sb, kT_sb[hp], hp, ni)
                    q_ch[(hp, ni)] = qk_chain(wq_sb, qT_sb[hp], hp, ni)
            for mj in range(NT):
                v_ch[mj] = v_chain(mj)

            # ---------------- attention emitters ----------------
            sT_live = {}
            pT_live = {}

            def emit_S(hp, col, mj):
                # producers first (program order == semantics)
                k_ch[(hp, mj // 4)].flush()
                q_ch[(hp, col // 512)].flush()
                sT = psum.tile([128, 1024], F32, name="sT", tag="sT", bufs=2)
                sT_live[(hp, col, mj)] = sT
                for i in range(2):
                    po = i * 64
                    nc.tensor.matmul(
                        sT[:, i * 512:(i + 1) * 512],
                        kT_sb[hp][po:po + 64, mj * 128:(mj + 1) * 128],
                        qT_sb[hp][po:po + 64, col:col + 512],
                        start=True, stop=True)

            def emit_evac(ci, hp, col, mj):
                sT = sT_live.pop((hp, col, mj))
                pT = work.tile([128, 1024], BF16, name="pT", tag="pT", bufs=16)
                pT_live[(hp, col, mj)] = pT
                if ci >= DVE_FROM_CHUNK and mj in DVE_SLOTS:
                    nc.vector.tensor_scalar(
                        pT[:].bitcast(I16), sT[:], float(EXP_A), float(EXP_B),
                        mybir.AluOpType.mult, mybir.AluOpType.add)
                else:
                    nc.scalar.activation(
                        pT[:], sT[:], mybir.ActivationFunctionType.Exp, scale=SCALE)

            def emit_O(hp, col, mj, oT):
                v_ch[mj].flush()
                pT = pT_live.pop((hp, col, mj))
                for i in range(2):
                    h = hp * 2 + i
                    vblk = v_sb[:, (mj * HPC + h) * 65:(mj * HPC + h) * 65 + 65]
                    nc.tensor.matmul(
                        oT[i][:], vblk, pT[:, i * 512:(i + 1) * 512],
                        start=(mj == 0), stop=(mj == NT - 1))

            def norm_head(hp, col, oT, rinvs):
                # free the oT psum banks fast: copy out + row-sum reciprocal
                for i in range(2):
                    po = i * 64
                    ao_slice = ao_sb[hp][po:po + 64, col:col + 512]
                    nc.vector.tensor_copy(ao_slice, oT[i][0:64, :])
                    r_row = small.tile([1, 512], F32, name="r_row", tag="r_row", bufs=4)
                    nc.vector.tensor_copy(r_row[:], oT[i][64:65, :])
                    rinv = small.tile([1, 512], F32, name="rinv", tag="rinv", bufs=4)
                    nc.vector.reciprocal_approx_fast(rinv[:], r_row[:])
                    rinvs.append(rinv)

            def norm_tail_chain(hp, col, g, rinvs):
                steps = []
                for i in range(2):
                    po = i * 64
                    rinv = rinvs[i]
                    st = {}
                    def s1(rinv=rinv, st=st):
                        rb_row = small.tile([1, 512], BF16, name="rb_row", tag="rb_row", bufs=4)
                        nc.vector.tensor_copy(rb_row[:], rinv[:])
                        rb_ps = psum.tile([64, 512], F32, name="rb_ps", tag="mm")
                        st['rb_ps'] = rb_ps
                        nc.tensor.matmul(rb_ps[:], ones_sb[:], rb_row[:], start=True, stop=True)
                    def s2(po=po, st=st):
                        ao_slice = ao_sb[hp][po:po + 64, col:col + 512]
                        nc.vector.tensor_mul(ao_slice, ao_slice, st['rb_ps'][:])
                        nc.gpsimd.dma_start(
                            ag_in[g][hp * 128 + po:hp * 128 + po + 64,
                                     col - AG_BASE[g]:col - AG_BASE[g] + 512],
                            ao_slice)
                    steps += [(240, s1), (30, s2)]
                return Chain(steps)

            # ---------------- prologue ----------------
            with nc.named_scope("qkv"):
                ka, qa = k_ch[(0, 0)], q_ch[(0, 0)]
                while ka.emit_next() is not None and qa.emit_next() is not None:
                    pass
                ka.flush()
                qa.flush()
                for mj in range(3):
                    v_ch[mj].flush()

            # pacing order for the rest (need() guarantees correctness)
            fillers.extend([k_ch[(0, 1)], v_ch[3], v_ch[4],
                            k_ch[(0, 2)], v_ch[5], v_ch[6],
                            k_ch[(0, 3)], v_ch[7], v_ch[8], v_ch[9],
                            q_ch[(0, 1)],
                            v_ch[10], v_ch[11], v_ch[12], v_ch[13], v_ch[14], v_ch[15]])
            for hp in (1, 2):
                for ni in range(4):
                    fillers.append(k_ch[(hp, ni)])
                for ni in range(4):
                    fillers.append(q_ch[(hp, ni)])
            fillers.append(q_ch[(0, 2)])
            fillers.append(q_ch[(0, 3)])

            # ---------------- main chunk loop ----------------
            group_tails = {0: [], 1: [], 2: []}
            pending_tail = None
            delayed = {}
            for ci, (hp, col, g) in enumerate(CHUNKS):
                with nc.named_scope(f"attn{ci}"):
                    if ci == 0:
                        emit_S(hp, col, 0)
                        emit_S(hp, col, 1)
                    if pending_tail is not None:
                        fillers.appendleft(pending_tail)
                        pending_tail = None
                    for ch in delayed.pop(ci, ()):
                        fillers.append(ch)
                    oT = [psum.tile([65, 512], F32, name=f"oT{i}", tag="oT")
                          for i in range(2)]
                    for mj in range(NT):
                        emit_evac(ci, hp, col, mj)
                        if mj + 2 < NT:
                            emit_S(hp, col, mj + 2)
                        elif ci + 1 < len(CHUNKS):
                            nhp, ncol, _ = CHUNKS[ci + 1]
                            emit_S(nhp, ncol, mj + 2 - NT)
                        emit_O(hp, col, mj, oT)
                        pump()
                    rinvs = []
                    norm_head(hp, col, oT, rinvs)
                    tail = norm_tail_chain(hp, col, g, rinvs)
                    group_tails[g].append(tail)
                    if ci in AG_AFTER:
                        gi = AG_AFTER[ci]
                        for ch in group_tails[gi]:
                            ch.flush()   # all ships of this group before the AG
                        with nc.named_scope(f"ag{gi}"):
                            nc.gpsimd.collective_compute(
                                "AllGather",
                                mybir.AluOpType.bypass,
                                replica_groups=[[0, 1], [2, 3], [4, 5], [6, 7]],
                                ins=[ag_in[gi].opt()],
                                outs=[ag_out[gi].opt()],
                            )
                        if gi == 0:
                            # pump during chunks ci+2.. (AG latency ~9us must
                            # pass before the aoF loads hit the sync queue);
                            # split so the MM load spreads over five chunks
                            delayed.setdefault(ci + 2, []).append(
                                proj_chain(0, njls=range(0, 4)))
                            delayed.setdefault(ci + 4, []).append(
                                proj_chain(0, njls=range(4, 8), load=False))
                        elif gi == 1:
                            # njl 10,11 held back: their MMs keep the PE warm
                            # during the final AllGather's latency window
                            delayed.setdefault(ci + 2, []).append(
                                proj_chain(1, njls=range(8, 9)))
                    else:
                        pending_tail = tail

            # ---------------- epilogue ----------------
            with nc.named_scope("proj2"):
                flush_fillers()
                # group-1 leftovers execute while AG2 is in flight (PE warm)
                proj_chain(1, njls=range(9, 12), load=False).flush()
                proj_chain(2, fast_load=True).flush()

    nc.finalize()
    return nc


_NC = None
LAST_RESULTS = None


def _get_nc():
    global _NC
    if _NC is None:
        _NC = _build()
    return _NC


def kernel(x, w_qkv, w_out, b_out, _trace=False):
    global LAST_RESULTS
    nc = _get_nc()

    x = np.asarray(x, dtype=np.float32)
    w_qkv = np.asarray(w_qkv, dtype=np.float32)
    w_out = np.asarray(w_out, dtype=np.float32)
    b_out = np.asarray(b_out, dtype=np.float32)

    bf16 = ml_dtypes.bfloat16
    in_maps = []
    for c in range(8):
        b, g = c // 2, c % 2
        s = g * KC
        in_maps.append({
            "xT": np.ascontiguousarray(x[b].T).astype(bf16),
            "wq": np.ascontiguousarray(w_qkv[:, s:s + KC]).astype(bf16),
            "wk": np.ascontiguousarray(w_qkv[:, C + s:C + s + KC]).astype(bf16),
            "wv": np.ascontiguousarray(w_qkv[:, 2 * C + s:2 * C + s + KC]).astype(bf16),
            "wo": np.ascontiguousarray(w_out[:, s:s + KC]).astype(bf16),
            "bb": np.tile(b_out[s:s + KC], (128, 1)),
        })

    res = run_bass_kernel_spmd(nc, in_maps, core_ids=list(range(8)), trace=_trace)
    LAST_RESULTS = res

    out = np.empty((B, N, C), dtype=np.float32)
    for c in range(8):
        b, g = c // 2, c % 2
        out[b, :, g * KC:(g + 1) * KC] = res.results[c]["y"].astype(np.float32)
    return out


# revision 22
# speedup vs baseline: 1.0278x; 1.0148x over previous
"""Multi-head attention (B=4, N=2048, C=768, H=12) on 8 TRN2 NeuronCores.

Sharding: 4 batches x 2 head-groups (6 heads each); core = 2*b + g.

Structure (v2): the kernel is ACT(exp)-bound -- 192 softmax tiles of
[128,1024] at ~1.11us each.  Everything else (qkv GEMMs, V, output
projection, collectives) is emitted as rate-limited "filler" chains pumped
into the attention slot loop so the PE's idle time absorbs them and the
ACT never starves.  Chains are demand-flushed (need()) before any consumer
instruction, so emission (program) order always respects data flow; the
pump is pacing only.  A fraction of the exp tiles can be offloaded to the
DVE via a Schraudolph int16 bit-trick (exp(x) ~= bitcast_bf16(int16(A*x+B)))
which runs concurrently with the ACT.

Per core:
  - qT/kT [64,2048] per head and v [2048,64] per head from host-pre-transposed xT
  - flash-style attention on transposed-S tiles:
      S^T(m,n) = kT.T @ qT   (PE, bf16, two heads in disjoint row groups)
      P^T = exp(S^T/8)       (ACT exp or DVE int16-Schraudolph, -> bf16)
      o^T = [v|1].T @ P^T    (PE; ones column accumulates softmax row-sums)
  - normalize columns of o^T via reciprocal + K=1 broadcast matmul
  - AllGather of normalized aoT (bf16) between pair cores in three column
    groups (1024/512/512) so the collectives overlap remaining attention
  - each core projects the gathered aoT onto its w_out column slice
Host only concatenates the 8 column-slices (no host math).
"""

import math
import sys
from collections import deque

sys.path.insert(0, "/opt/trn_rl_repo")

import ml_dtypes
import numpy as np

import concourse.bass as bass
import concourse.mybir as mybir
from concourse import bacc, tile
from concourse.bass_utils import run_bass_kernel_spmd

F32 = mybir.dt.float32
BF16 = mybir.dt.bfloat16
I16 = mybir.dt.int16

B, N, C, H, D = 4, 2048, 768, 12, 64
G = 2               # head groups (tensor-parallel dim)
HPC = H // G        # heads per core = 6
KC = HPC * D        # per-core head width = 384
CT = C // 128       # contraction tiles over C = 6
NT = N // 128       # 128-row seq tiles = 16
SCALE = D ** -0.5

# Schraudolph exp-in-bf16-bits: exp(s*SCALE) ~= bitcast_bf16(int16(A*s + B)).
# C16 tuned offline (numpy sim: rel err 1.2e-2 at 25% offload vs 5.3e-3 pure;
# gate is 2e-2).
_C16 = 5.5
EXP_A = (128.0 / math.log(2.0)) * SCALE
EXP_B = 127.0 * 128.0 - _C16
# mj slots (0..15 within each chunk) evacuated on DVE instead of ACT
DVE_SLOTS = frozenset()  # phase 1 validation: ACT-only
DVE_FROM_CHUNK = 1  # skip chunk 0 (DVE busy with v casts / qk fillers)

# chunk list: (hp, col, ag_idx); col = 512-wide q-column base
CHUNKS = [
    (0, 0, 0), (0, 512, 0), (1, 0, 0), (1, 512, 0), (2, 0, 0), (2, 512, 0),
    (0, 1024, 1), (1, 1024, 1), (2, 1024, 1),
    (0, 1536, 2), (1, 1536, 2), (2, 1536, 2),
]
AG_AFTER = {5: 0, 8: 1, 11: 2}          # chunk idx -> ag group emitted after
AG_BASE = {0: 0, 1: 1024, 2: 1536}      # ag group -> q-column base
AG_W = {0: 1024, 1: 512, 2: 512}        # ag group -> width
AG_NJL = {0: range(0, 8), 1: range(8, 12), 2: range(12, 16)}  # y 128-row blocks


class Chain:
    """A short sequence of emission steps with demand-flush semantics."""

    __slots__ = ("steps", "idx")

    def __init__(self, steps):
        self.steps = steps
        self.idx = 0

    def emit_next(self):
        if self.idx >= len(self.steps):
            return None
        cost, fn = self.steps[self.idx]
        self.idx += 1
        fn()
        return cost

    def flush(self):
        while self.emit_next() is not None:
            pass


def _build():
    nc = bacc.Bacc(None, num_devices=8)

    xT_d = nc.declare_dram_parameter("xT", [C, N], BF16, isOutput=False)
    wq_d = nc.declare_dram_parameter("wq", [C, KC], BF16, isOutput=False)
    wk_d = nc.declare_dram_parameter("wk", [C, KC], BF16, isOutput=False)
    wv_d = nc.declare_dram_parameter("wv", [C, KC], BF16, isOutput=False)
    wo_d = nc.declare_dram_parameter("wo", [C, KC], BF16, isOutput=False)
    bb_d = nc.declare_dram_parameter("bb", [128, KC], F32, isOutput=False)
    y_d = nc.declare_dram_parameter("y", [N, KC], BF16, isOutput=True)

    with tile.TileContext(nc) as tc:
        with (
            tc.tile_pool(name="wpool", bufs=1) as wpool,
            tc.tile_pool(name="work", bufs=3) as work,
            tc.tile_pool(name="psum", bufs=2, space="PSUM") as psum,
            tc.tile_pool(name="dram", bufs=1, space="DRAM") as dram,
        ):
            xpool = seq = wpool
            small = work
            # ---- SBUF residents ----
            wq_sb = wpool.tile([128, CT, KC], BF16)
            wk_sb = wpool.tile([128, CT, KC], BF16)
            wv_sb = wpool.tile([128, CT, KC], BF16)
            wo_sb = wpool.tile([128, CT, KC], BF16)
            bb_sb = wpool.tile([128, KC], F32)
            xT_sb = xpool.tile([128, CT, N], BF16)
            qT_sb = [seq.tile([128, N], BF16, name=f"qT{t}", tag=f"qT{t}") for t in range(3)]
            kT_sb = [seq.tile([128, N], BF16, name=f"kT{t}", tag=f"kT{t}") for t in range(3)]
            v_sb = seq.tile([128, NT * HPC * 65], BF16, tag="v")
            ao_sb = [seq.tile([128, N], BF16, name=f"ao{t}", tag=f"ao{t}") for t in range(3)]
            ones_sb = small.tile([1, 64], BF16, bufs=1)

            with nc.named_scope("load"):
                # Two parallel DMA rings (sync + gpsimd); wk/wq/xT0 first --
                # the prologue's k/q/v ni=0 GEMMs gate the first attention slot.
                # (Do NOT trigger DMAs from the scalar queue: each trigger costs
                # ~650ns of ACT-engine time and starves the exp pipeline.)
                for ct in range(CT):
                    nc.sync.dma_start(wk_sb[:, ct, :], wk_d[ct * 128:(ct + 1) * 128, :])
                    nc.sync.dma_start(wq_sb[:, ct, :], wq_d[ct * 128:(ct + 1) * 128, :])
                    nc.gpsimd.dma_start(xT_sb[:, ct, 0:512], xT_d[ct * 128:(ct + 1) * 128, 0:512])
                for ct in range(CT):
                    nc.gpsimd.dma_start(wv_sb[:, ct, :], wv_d[ct * 128:(ct + 1) * 128, :])
                for ct in range(CT):
                    nc.gpsimd.dma_start(
                        xT_sb[:, ct, 512:1024], xT_d[ct * 128:(ct + 1) * 128, 512:1024])
                for ni in range(2, 4):
                    for ct in range(CT):
                        nc.sync.dma_start(
                            xT_sb[:, ct, ni * 512:(ni + 1) * 512],
                            xT_d[ct * 128:(ct + 1) * 128, ni * 512:(ni + 1) * 512],
                        )
                for ct in range(CT):
                    nc.sync.dma_start(wo_sb[:, ct, :], wo_d[ct * 128:(ct + 1) * 128, :])
                nc.sync.dma_start(bb_sb[:], bb_d[:])

            # ones column at offset 64 of every 65-wide v block (row-sum trick)
            nc.vector.memset(v_sb.rearrange("p (b s) -> p b s", s=65)[:, :, 64], 1.0)
            nc.vector.memset(ones_sb[:], 1.0)

            ag_in = [dram.tile([KC, AG_W[g]], BF16, name=f"ag_in{g}") for g in range(3)]
            ag_out = [dram.tile([C, AG_W[g]], BF16, name=f"ag_out{g}") for g in range(3)]

            # ---------------- filler chains ----------------
            # The deque paces opportunistic emission into PE slack; need()
            # force-flushes a producer chain before its consumer is emitted.
            fillers = deque()
            pump_state = {'credit': 0}

            def pump(budget=400):
                credit = min(pump_state['credit'] + budget, 1000)
                while fillers and credit > 0:
                    cost = fillers[0].emit_next()
                    if cost is None:
                        fillers.popleft()
                        continue
                    credit -= cost
                pump_state['credit'] = credit if fillers else 0

            def flush_fillers():
                while fillers:
                    fillers.popleft().flush()

            def qk_chain(wsb, dst, hp, ni):
                """3 steps; projects 512 seq-cols of q or k for head-pair hp."""
                st = {}
                def mk(cts, first, last):
                    def s():
                        if first:
                            st['ps'] = psum.tile([128, 512], F32, name="qk_ps", tag="mm")
                        for ct in cts:
                            nc.tensor.matmul(
                                st['ps'][:], wsb[:, ct, hp * 128:(hp + 1) * 128],
                                xT_sb[:, ct, ni * 512:(ni + 1) * 512],
                                start=(first and ct == cts[0]), stop=(last and ct == cts[-1]))
                        if last:
                            nc.vector.tensor_copy(dst[:, ni * 512:(ni + 1) * 512], st['ps'][:])
                    return s
                return Chain([(440, mk((0, 1), True, False)),
                              (440, mk((2, 3), False, False)),
                              (470, mk((4, 5), False, True))])

            def v_chain(mj):
                """3 steps; v rows for seq-tile mj, all 6 heads (65-strided)."""
                st = {}
                def mk(cts, first, last):
                    def s():
                        if first:
                            st['ps'] = psum.tile([128, KC], F32, name="v_ps", tag="mm")
                        for ct in cts:
                            nc.tensor.matmul(
                                st['ps'][:], xT_sb[:, ct, mj * 128:(mj + 1) * 128],
                                wv_sb[:, ct, :],
                                start=(first and ct == cts[0]), stop=(last and ct == cts[-1]))
                        if last:
                            dst = v_sb.rearrange("p (b s) -> p b s", s=65)[
                                :, mj * HPC:(mj + 1) * HPC, 0:64]
                            src = st['ps'][:].rearrange("p (h d) -> p h d", d=64)
                            nc.vector.tensor_copy(dst, src)
                    return s
                return Chain([(340, mk((0, 1), True, False)),
                              (340, mk((2, 3), False, False)),
                              (370, mk((4, 5), False, True))])

            aoF = [work.tile([128, 1024], BF16, name=f"aoF{kt}", tag=f"aoF{kt}", bufs=1)
                   for kt in range(CT)]

            def proj_chain(g, njls=None, load=True, fast_load=False):
                """aoF loads + y projection for ag group g (after its AllGather)."""
                w = AG_W[g]
                steps = []
                if load:
                    for kt in range(CT):
                        def sdma(kt=kt):
                            # fast_load: split across two idle queues (epilogue)
                            eng = nc.gpsimd if (fast_load and kt % 2) else nc.sync
                            eng.dma_start(aoF[kt][:, 0:w], ag_out[g][kt * 128:(kt + 1) * 128, :])
                        steps.append((30, sdma))
                for nj in (AG_NJL[g] if njls is None else njls):
                    cbase = nj * 128 - AG_BASE[g]
                    st = {}
                    def mk(kts, first, last, nj=nj, cbase=cbase, st=st):
                        def s():
                            if first:
                                st['ps'] = psum.tile([128, KC], F32, name="y_ps", tag="mm")
                            for kt in kts:
                                nc.tensor.matmul(
                                    st['ps'][:], aoF[kt][:, cbase:cbase + 128],
                                    wo_sb[:, kt, :],
                                    start=(first and kt == kts[0]),
                                    stop=(last and kt == kts[-1]))
                            if last:
                                y_sb = work.tile([128, KC], BF16, name="y_sb", tag="y")
                                nc.vector.tensor_add(y_sb[:], st['ps'][:], bb_sb[:])
                                nc.sync.dma_start(y_d[nj * 128:(nj + 1) * 128, :], y_sb[:])
                        return s
                    steps += [(510, mk((0, 1, 2), True, False)),
                              (540, mk((3, 4, 5), False, True))]
                return Chain(steps)

            # chain registries for demand flushing
            k_ch = {}
            q_ch = {}
            v_ch = {}
            for hp in range(3):
                for ni in range(4):
                    k_ch[(hp, ni)] = qk_chain(wk_sb, kT_sb[hp], hp, ni)
                    q_ch[(hp, ni)] = qk_chain(wq_sb, qT_sb[hp], hp, ni)
            for mj in range(NT):
                v_ch[mj] = v_chain(mj)

            # ---------------- attention emitters ----------------
            sT_live = {}
            pT_live = {}

            def emit_S(hp, col, mj):
                # producers first (program order == semantics)
                k_ch[(hp, mj // 4)].flush()
                q_ch[(hp, col // 512)].flush()
                sT = psum.tile([128, 1024], F32, name="sT", tag="sT", bufs=2)
                sT_live[(hp, col, mj)] = sT
                for i in range(2):
                    po = i * 64
                    nc.tensor.matmul(
                        sT[:, i * 512:(i + 1) * 512],
                        kT_sb[hp][po:po + 64, mj * 128:(mj + 1) * 128],
                        qT_sb[hp][po:po + 64, col:col + 512],
                        start=True, stop=True)

            def emit_evac(ci, hp, col, mj):
                sT = sT_live.pop((hp, col, mj))
                pT = work.tile([128, 1024], BF16, name="pT", tag="pT", bufs=16)
                pT_live[(hp, col, mj)] = pT
                if ci >= DVE_FROM_CHUNK and mj in DVE_SLOTS:
                    nc.vector.tensor_scalar(
                        pT[:].bitcast(I16), sT[:], float(EXP_A), float(EXP_B),
                        mybir.AluOpType.mult, mybir.AluOpType.add)
                else:
                    nc.scalar.activation(
                        pT[:], sT[:], mybir.ActivationFunctionType.Exp, scale=SCALE)

            def emit_O(hp, col, mj, oT):
                v_ch[mj].flush()
                pT = pT_live.pop((hp, col, mj))
                for i in range(2):
                    h = hp * 2 + i
                    vblk = v_sb[:, (mj * HPC + h) * 65:(mj * HPC + h) * 65 + 65]
                    nc.tensor.matmul(
                        oT[i][:], vblk, pT[:, i * 512:(i + 1) * 512],
                        start=(mj == 0), stop=(mj == NT - 1))

            def norm_head(hp, col, oT, rinvs):
                # free the oT psum banks fast: copy out + row-sum reciprocal
                for i in range(2):
                    po = i * 64
                    ao_slice = ao_sb[hp][po:po + 64, col:col + 512]
                    nc.vector.tensor_copy(ao_slice, oT[i][0:64, :])
                    r_row = small.tile([1, 512], F32, name="r_row", tag="r_row", bufs=4)
                    nc.vector.tensor_copy(r_row[:], oT[i][64:65, :])
                    rinv = small.tile([1, 512], F32, name="rinv", tag="rinv", bufs=4)
                    nc.vector.reciprocal_approx_fast(rinv[:], r_row[:])
                    rinvs.append(rinv)

            def norm_tail_chain(hp, col, g, rinvs):
                steps = []
                for i in range(2):
                    po = i * 64
                    rinv = rinvs[i]
                    st = {}
                    def s1(rinv=rinv, st=st):
                        rb_row = small.tile([1, 512], BF16, name="rb_row", tag="rb_row", bufs=4)
                        nc.vector.tensor_copy(rb_row[:], rinv[:])
                        rb_ps = psum.tile([64, 512], F32, name="rb_ps", tag="mm")
                        st['rb_ps'] = rb_ps
                        nc.tensor.matmul(rb_ps[:], ones_sb[:], rb_row[:], start=True, stop=True)
                    def s2(po=po, st=st):
                        ao_slice = ao_sb[hp][po:po + 64, col:col + 512]
                        nc.vector.tensor_mul(ao_slice, ao_slice, st['rb_ps'][:])
                        nc.gpsimd.dma_start(
                            ag_in[g][hp * 128 + po:hp * 128 + po + 64,
                                     col - AG_BASE[g]:col - AG_BASE[g] + 512],
                            ao_slice)
                    steps += [(240, s1), (30, s2)]
                return Chain(steps)

            # ---------------- prologue ----------------
            with nc.named_scope("qkv"):
                ka, qa = k_ch[(0, 0)], q_ch[(0, 0)]
                while ka.emit_next() is not None and qa.emit_next() is not None:
                    pass
                ka.flush()
                qa.flush()
                for mj in range(3):
                    v_ch[mj].flush()

            # pacing order for the rest (need() guarantees correctness)
            fillers.extend([k_ch[(0, 1)], v_ch[3], v_ch[4],
                            k_ch[(0, 2)], v_ch[5], v_ch[6],
                            k_ch[(0, 3)], v_ch[7], v_ch[8], v_ch[9],
                            q_ch[(0, 1)],
                            v_ch[10], v_ch[11], v_ch[12], v_ch[13], v_ch[14], v_ch[15]])
            for hp in (1, 2):
                for ni in range(4):
                    fillers.append(k_ch[(hp, ni)])
                for ni in range(4):
                    fillers.append(q_ch[(hp, ni)])
            fillers.append(q_ch[(0, 2)])
            fillers.append(q_ch[(0, 3)])

            # ---------------- main chunk loop ----------------
            group_tails = {0: [], 1: [], 2: []}
            pending_tail = None
            delayed = {}
            for ci, (hp, col, g) in enumerate(CHUNKS):
                with nc.named_scope(f"attn{ci}"):
                    if ci == 0:
                        emit_S(hp, col, 0)
                        emit_S(hp, col, 1)
                    if pending_tail is not None:
                        fillers.appendleft(pending_tail)
                        pending_tail = None
                    for ch in delayed.pop(ci, ()):
                        fillers.append(ch)
                    oT = [psum.tile([65, 512], F32, name=f"oT{i}", tag="oT")
                          for i in range(2)]
                    for mj in range(NT):
                        emit_evac(ci, hp, col, mj)
                        if mj + 2 < NT:
                            emit_S(hp, col, mj + 2)
                        elif ci + 1 < len(CHUNKS):
                            nhp, ncol, _ = CHUNKS[ci + 1]
                            emit_S(nhp, ncol, mj + 2 - NT)
                        emit_O(hp, col, mj, oT)
                        pump()
                    rinvs = []
                    norm_head(hp, col, oT, rinvs)
                    tail = norm_tail_chain(hp, col, g, rinvs)
                    group_tails[g].append(tail)
                    if ci in AG_AFTER:
                        gi = AG_AFTER[ci]
                        for ch in group_tails[gi]:
                            ch.flush()   # all ships of this group before the AG
                        with nc.named_scope(f"ag{gi}"):
                            nc.gpsimd.collective_compute(
                                "AllGather",
                                mybir.AluOpType.bypass,
                                replica_groups=[[0, 1], [2, 3], [4, 5], [6, 7]],
                                ins=[ag_in[gi].opt()],
                                outs=[ag_out[gi].opt()],
                            )
                        if gi == 0:
                            # pump during chunks ci+2.. (AG latency ~9us must
                            # pass before the aoF loads hit the sync queue);
                            # split so the MM load spreads over five chunks
                            delayed.setdefault(ci + 2, []).append(
                                proj_chain(0, njls=range(0, 4)))
                            delayed.setdefault(ci + 4, []).append(
                                proj_chain(0, njls=range(4, 8), load=False))
                        elif gi == 1:
                            # njl 10,11 held back: their MMs keep the PE warm
                            # during the final AllGather's latency window
                            delayed.setdefault(ci + 2, []).append(
                                proj_chain(1, njls=range(8, 9)))
                    else:
                        pending_tail = tail

            # ---------------- epilogue ----------------
            with nc.named_scope("proj2"):
                flush_fillers()
                # group-1 leftovers execute while AG2 is in flight (PE warm)
                proj_chain(1, njls=range(9, 12), load=False).flush()
                proj_chain(2, fast_load=True).flush()

    nc.finalize()
    return nc


_NC = None
LAST_RESULTS = None


def _get_nc():
    global _NC
    if _NC is None:
        _NC = _build()
    return _NC


def kernel(x, w_qkv, w_out, b_out, _trace=False):
    global LAST_RESULTS
    nc = _get_nc()

    x = np.asarray(x, dtype=np.float32)
    w_qkv = np.asarray(w_qkv, dtype=np.float32)
    w_out = np.asarray(w_out, dtype=np.float32)
    b_out = np.asarray(b_out, dtype=np.float32)

    bf16 = ml_dtypes.bfloat16
    in_maps = []
    for c in range(8):
        b, g = c // 2, c % 2
        s = g * KC
        in_maps.append({
            "xT": np.ascontiguousarray(x[b].T).astype(bf16),
            "wq": np.ascontiguousarray(w_qkv[:, s:s + KC]).astype(bf16),
            "wk": np.ascontiguousarray(w_qkv[:, C + s:C + s + KC]).astype(bf16),
            "wv": np.ascontiguousarray(w_qkv[:, 2 * C + s:2 * C + s + KC]).astype(bf16),
            "wo": np.ascontiguousarray(w_out[:, s:s + KC]).astype(bf16),
            "bb": np.tile(b_out[s:s + KC], (128, 1)),
        })

    res = run_bass_kernel_spmd(nc, in_maps, core_ids=list(range(8)), trace=_trace)
    LAST_RESULTS = res

    out = np.empty((B, N, C), dtype=np.float32)
    for c in range(8):
        b, g = c // 2, c % 2
        out[b, :, g * KC:(g + 1) * KC] = res.results[c]["y"].astype(np.float32)
    return out
